# revision 11
# baseline (speedup 1.0000x reference)
"""LSTM encoder kernel for Trainium2 (Bass/Tile), data-parallel over batch on 8 cores.

Math (per core, batch shard B=256):
  z_t = Wcat @ [x_t ; hh_{t-1}] + b      (gates pre-activation, [128, B])
  Wcat = [Wx ; 2*Whh] with g-gate columns additionally scaled by 2 so a single
  sigmoid over gate rows yields S_g = sigmoid(2 z_g), i.e.
  tanh(z_g) = 2 S_g - 1.  Reparametrize cc = c/2, hh = h/2:
    u  = (S_g - 1/2) * S_i = (i*g)/2
    v  = S_f * cc          = (f*c)/2
    cc = v + u             = c_new/2
    S_c = sigmoid(4*cc)    = sigmoid(2*c_new)
    hh = (S_c - 1/2) * S_o = o*tanh(c_new)/2 = h/2
  Host multiplies the stored hh history by 2 to recover h.

Gate order in Wcat columns is [f, i, o, g] (partitions 0:32, 32:64, 64:96,
96:128 of the PSUM gates). Two activations: sigma_fio covers rows 0:96 in
place; sigma_g relocates rows 96:128 -> partitions 32:64 so that
u = (S_g-0.5)*S_i is a single STT with both SBUF operands at base partition 32
(HW requires equal base partitions for two-SBUF-operand DVE ops).
v runs on DVE under the sigma_g window; u,cc follow; sigma_c relocates
cc (0:32) -> 64:96 to pair with S_o for the hh STT. Matmul operands fp16
(fp32 would be a dual-pass LOW/HIGH matmul, ~700ns more in the chain).

Layouts: gates on partitions (128), batch on free dim. Per chunk of TC steps one
SBUF tile [42, TC*B] holds rhs slots [hh_{t-1} ; x_t]; the hh write of step t
lands in slot t+1 (next chunk's slot 0 at boundaries). Output DMA reads rows
0:32. Two batch blocks (fd=128) run as rotated software pipelines.
"""

import numpy as np
from contextlib import ExitStack

import concourse.bass as bass
import concourse.tile as tile
from concourse import bacc, mybir
from concourse.bass_utils import run_bass_kernel_spmd

T_FULL = 512
B_FULL = 2048
IN = 10
H = 32
G = 4 * H          # 128 gate rows
K = IN + H         # 42 contraction rows of the combined matmul
NCORES = 8
B = B_FULL // NCORES  # 256 batch per core

NB = 2          # batch sub-blocks per core (latency pipelining)
FD = B // NB    # free-dim per block
TC = 16         # timesteps per SBUF chunk

DT = mybir.dt.float32
DTM = mybir.dt.float16   # matmul operand dtype (single-pass PE, vs fp32 dual-pass)
SIG = mybir.ActivationFunctionType.Sigmoid
MULT = mybir.AluOpType.mult
ADD = mybir.AluOpType.add
SUB = mybir.AluOpType.subtract

_CACHE = {}


def _build(t_total=T_FULL, tc=TC, nb=NB):
    fd = B // nb
    nchunk = t_total // tc
    nc = bacc.Bacc(trn_type="TRN2", debug=False, target_bir_lowering=False)

    xT = nc.dram_tensor("xT", [t_total, IN, B], DTM, kind="ExternalInput").ap()
    wcat = nc.dram_tensor("wcat", [K, G], DTM, kind="ExternalInput").ap()
    bg = nc.dram_tensor("bg", [G, 1], DT, kind="ExternalInput").ap()
    hout = nc.dram_tensor("hout", [t_total, H, B], DTM, kind="ExternalOutput").ap()

    with tile.TileContext(nc) as tc_, ExitStack() as ctx:
        const = ctx.enter_context(tc_.tile_pool(name="const", bufs=1))
        xpool = ctx.enter_context(tc_.tile_pool(name="xpool", bufs=3))
        spool = ctx.enter_context(tc_.tile_pool(name="spool", bufs=8))
        cpool = ctx.enter_context(tc_.tile_pool(name="cpool", bufs=4))
        tpool = ctx.enter_context(tc_.tile_pool(name="tpool", bufs=12))
        pspool = ctx.enter_context(tc_.tile_pool(name="pspool", bufs=4, space="PSUM"))

        w_t = const.tile([K, G], DTM)
        nc.sync.dma_start(w_t[:], wcat)
        bg_t = const.tile([G, 1], DT)
        nc.sync.dma_start(bg_t[:], bg)

        # rhs chunk tiles: [K, tc*B]; rows 0:H = hh slots, rows H:K = x slots
        chunk_tiles = {}

        def get_chunk(ch):
            if ch not in chunk_tiles:
                t = xpool.tile([K, tc * B], DTM, name="rhs", tag="rhs")
                if ch < nchunk:
                    nc.sync.dma_start(
                        t[H:K].rearrange("p (t b) -> p t b", t=tc),
                        xT[ch * tc:(ch + 1) * tc].rearrange("t p b -> p t b"),
                    )
                chunk_tiles[ch] = t
            return chunk_tiles[ch]

        cur = get_chunk(0)
        # hh_{-1} = 0
        nc.vector.memset(cur[0:H, 0:B], 0.0)

        c_prev = []
        for blk in range(nb):
            c0 = cpool.tile([H, fd], DT, name=f"cc{blk}", tag=f"cc{blk}")
            nc.vector.memset(c0[:], 0.0)
            c_prev.append(c0)

        # Rotated software pipeline: block 1 runs half a step behind block 0.
        # Phase A(b, s): mm -> sigma_fio -> sigma_g (relocate) ; v under sigma_g
        # Phase B(b, s): u -> cc -> sigma_c -> hh
        # Tick s: A(b1, s), B(b0, s), A(b0, s+1), B(b1, s)
        state = {}

        def phase_a(blk, s_global):
            ch_, s_ = divmod(s_global, tc)
            col = s_ * B + blk * fd
            rhs = get_chunk(ch_)
            p = pspool.tile([G, fd], mybir.dt.float32, name="gates",
                            tag=f"gates{blk}")
            nc.tensor.matmul(p[:], w_t[:], rhs[:, col:col + fd],
                             start=True, stop=True)
            # sigma over [f,i,o] rows in place
            s_t = spool.tile([3 * H, fd], DT, name="sgm", tag=f"sgm{blk}")
            nc.scalar.activation(s_t[:], p[0:3 * H], SIG, bias=bg_t[0:3 * H])
            # sigma over g rows, relocated to partitions 32:64 (pairs with S_i)
            gt = tpool.tile([2 * H, fd], DT, name="sg", tag=f"sg{blk}")
            nc.scalar.activation(gt[H:2 * H], p[3 * H:4 * H], SIG,
                                 bias=bg_t[3 * H:4 * H])
            # v = f * cc (start 0), hidden under the sigma_g window
            v = tpool.tile([H, fd], DT, name="v", tag=f"v{blk}")
            nc.vector.tensor_tensor(v[:], s_t[0:H], c_prev[blk][:], MULT)
            state[blk] = (s_t, gt, v, s_global)

        def phase_b(blk):
            s_t, gt, v, s_global = state[blk]
            # u = (S_g - 0.5) * S_i, both operands at base partition 32
            u = tpool.tile([H, fd], DT, name="u", tag=f"u{blk}")
            nc.vector.scalar_tensor_tensor(
                u[:], gt[H:2 * H], 0.5, s_t[H:2 * H], SUB, MULT)
            c_new = cpool.tile([H, fd], DT, name=f"ccn{blk}", tag=f"cc{blk}")
            nc.vector.tensor_tensor(c_new[:], v[:], u[:], ADD)
            c_prev[blk] = c_new
            # sc = sigmoid(4*cc) relocated to 64:96 (pairs with o)
            sc = spool.tile([3 * H, fd], DT, name="sc", tag=f"sc{blk}")
            nc.scalar.activation(sc[2 * H:3 * H], c_new[:], SIG, scale=4.0)
            ch_, s_ = divmod(s_global + 1, tc)
            col = s_ * B + blk * fd
            hdst = get_chunk(ch_)[0:H, col:col + fd]
            nc.vector.scalar_tensor_tensor(
                hdst, sc[2 * H:3 * H], 0.5, s_t[2 * H:3 * H], SUB, MULT)

        def emit_out(ch):
            cur_, nxt_ = get_chunk(ch), get_chunk(ch + 1)
            nc.sync.dma_start(
                hout[ch * tc:ch * tc + tc - 1].rearrange("t p b -> p t b"),
                cur_[0:H, B:].rearrange("p (t b) -> p t b", t=tc - 1),
            )
            nc.sync.dma_start(hout[ch * tc + tc - 1], nxt_[0:H, 0:B])

        phase_a(0, 0)
        for s in range(t_total):
            phase_a(1, s)
            phase_b(0)
            if s + 1 < t_total:
                phase_a(0, s + 1)
            phase_b(1)
            if s % tc == tc - 1:
                emit_out(s // tc)
    nc.compile()
    return nc


def _prep_weights(W_emb, b_emb, W_ih, W_hh, b_ih, b_hh):
    f8 = lambda a: np.asarray(a, np.float64)
    Wx = f8(W_ih) @ f8(W_emb)                                  # [G, IN]
    bgv = f8(W_ih) @ f8(b_emb) + f8(b_ih) + f8(b_hh)           # [G]
    wc = np.concatenate([2.0 * f8(W_hh).T, Wx.T], axis=0)      # [K, G] = [hh; x]
    # reorder gate columns [i,f,g,o] -> [f,i,o,g]; scale g block (now last) by 2
    perm = np.concatenate([np.arange(H, 2 * H), np.arange(0, H),
                           np.arange(3 * H, 4 * H), np.arange(2 * H, 3 * H)])
    wc = wc[:, perm]
    bgv = bgv[perm]
    wc[:, 3 * H:4 * H] *= 2.0
    bgv[3 * H:4 * H] *= 2.0
    return (np.ascontiguousarray(wc.astype(np.float16)),
            np.ascontiguousarray(bgv.astype(np.float32).reshape(G, 1)))


def _run(x, W_emb, b_emb, W_ih, W_hh, b_ih, b_hh, trace=False):
    t_total = x.shape[0]
    key = (t_total, TC, NB)
    if key not in _CACHE:
        _CACHE[key] = _build(t_total, TC, NB)
    nc = _CACHE[key]

    wc, bgv = _prep_weights(W_emb, b_emb, W_ih, W_hh, b_ih, b_hh)
    x = np.asarray(x, np.float32)
    in_maps = []
    for c in range(NCORES):
        xs = np.ascontiguousarray(
            x[:, c * B:(c + 1) * B, :].transpose(0, 2, 1).astype(np.float16))
        in_maps.append({"xT": xs, "wcat": wc, "bg": bgv})

    res = run_bass_kernel_spmd(nc, in_maps, list(range(NCORES)), trace=trace)
    out = np.empty((t_total, B_FULL, H), np.float32)
    for c in range(NCORES):
        out[:, c * B:(c + 1) * B, :] = (
            res.results[c]["hout"].astype(np.float32).transpose(0, 2, 1)
            * np.float32(2.0))
    return out, res


def kernel(x, W_emb, b_emb, W_ih, W_hh, b_ih, b_hh):
    out, _ = _run(x, W_emb, b_emb, W_ih, W_hh, b_ih, b_hh, trace=False)
    return out
